# revision 1
# baseline (speedup 1.0000x reference)
"""AV temporal gated-conv MLP block for Trainium2 (8 NeuronCores, Bass/Tile).

Per-core strategy: pure data parallelism over the batch (B=8 -> 1 batch
element per core, both modalities on the same core since the gating couples
them). No collectives. Per core, loop over 4 l-blocks of 512 tokens:

  T: rms-norm in natural [l, d] layout -- ACT Square with accum_out for the
     square-sums (batched so ACT stays in one spline-table set), then rsqrt
     entirely on DVE (bit-trick seed + 3 Newton iterations, fp32-exact),
     per-partition scale, and PE-transpose of x_n to [d, l] (fp32r
     transpose mode, 4 tiles packed per PSUM bank, strided eviction).
  A: in_proj as fp32r matmuls (full 1 cycle/row PE rate) producing [e, l]
     chunks; causal depthwise conv on the x-half as 1 tensor_scalar + 3
     scalar_tensor_tensor fused MACs on the otherwise-idle VectorE; silu
     (ACT, Silu table set) on the w-half straight out of PSUM; cross-modal
     gate fused with the conv bias via scalar_tensor_tensor
     ((conv + b[ch]) * silu) writing the gated activations.
  B: out_proj fp32r matmuls in c-outer order (weight tiles stream through a
     4-slot pool; B's chunk c only needs gated chunk c so it can chase A),
     residual add on DVE, DMA out.

Weights are host-side packed: norm weights folded into in_proj, transposed
and m-tile-blocked so every weight DMA is a single contiguous read. Only
Square/Silu/Copy run on ACT => a single activation-table load for the whole
kernel (Sqrt would force 20+ table reloads).

Engine budget per core (cost model): PE 678us (matmul floor 654us), DMA
699us (252MB at ~358GB/s -- weight restreaming is SBUF-bound), DVE 444us,
ACT 218us; end-to-end ~769us predicted, ~850-920us measured through the
axon proxy. The T-chain priority boost (tc.high_priority) pulls each
block's load+square chain ahead of the previous block's B-phase traffic.
"""
import sys

if "/opt/trn_rl_repo" not in sys.path:
    sys.path.insert(0, "/opt/trn_rl_repo")

import numpy as np

DIM = 1024
INNER = 2048
L = 2048
B = 8
NCORES = 8
EPS = 1e-5
LB = 512              # l-block (tokens per block)
NB = L // LB          # 4 blocks
NXC = INNER // 128    # 16 x-half e-chunks per modality
NKD = DIM // 128      # 8 contraction chunks for in_proj
NLT = LB // 128       # 4 l-tiles per block
NN = DIM // 512       # 2 out_proj n-tiles
CONV_MODE = "dve"  # "pe" | "dve" | "gps" | "dve+gps" | "pe+dve"

_cache = {}


def _build_nc(conv_mode=None, repeat=1):
    conv_mode = conv_mode or CONV_MODE
    from contextlib import ExitStack

    import concourse.bass as bass
    import concourse.tile as tile
    from concourse import bacc, mybir
    from concourse.masks import make_identity

    dt = mybir.dt
    f32 = dt.float32
    f32r = dt.float32r
    bf16 = dt.bfloat16
    i32 = dt.int32
    AOP = mybir.AluOpType
    AF = mybir.ActivationFunctionType

    nc = bacc.Bacc("TRN2", target_bir_lowering=False, debug=False,
                   num_devices=NCORES)

    x_dram = {
        "a": nc.dram_tensor("xa", [L, DIM], f32, kind="ExternalInput").ap(),
        "v": nc.dram_tensor("xv", [L, DIM], f32, kind="ExternalInput").ap(),
    }
    win_dram = {
        "a": nc.dram_tensor("wina", [2 * NXC, 128, NKD, 128], f32r,
                            kind="ExternalInput").ap(),
        "v": nc.dram_tensor("winv", [2 * NXC, 128, NKD, 128], f32r,
                            kind="ExternalInput").ap(),
    }
    wout_dram = {
        "a": nc.dram_tensor("wouta", [NXC, NN, 128, 512], f32r,
                            kind="ExternalInput").ap(),
        "v": nc.dram_tensor("woutv", [NXC, NN, 128, 512], f32r,
                            kind="ExternalInput").ap(),
    }
    cw_dram = {
        "a": nc.dram_tensor("cwa", [128, NXC * 4], f32, kind="ExternalInput").ap(),
        "v": nc.dram_tensor("cwv", [128, NXC * 4], f32, kind="ExternalInput").ap(),
    }
    cbc_dram = {
        "a": nc.dram_tensor("cbca", [128, NXC], f32, kind="ExternalInput").ap(),
        "v": nc.dram_tensor("cbcv", [128, NXC], f32, kind="ExternalInput").ap(),
    }
    y = nc.dram_tensor("y", [2 * L, DIM], f32, kind="ExternalOutput").ap()

    MODS = ("a", "v")

    with tile.TileContext(nc) as tc, ExitStack() as ctx:
        sing = ctx.enter_context(tc.tile_pool(name="sing", bufs=1))
        p_xT = ctx.enter_context(tc.tile_pool(name="xT", bufs=2))
        p_gat = ctx.enter_context(tc.tile_pool(name="gat", bufs=2))
        p_xin = ctx.enter_context(tc.tile_pool(name="xin", bufs=5))
        p_xn = ctx.enter_context(tc.tile_pool(name="xn", bufs=5))
        p_stat = ctx.enter_context(tc.tile_pool(name="stat", bufs=4))
        p_win = ctx.enter_context(tc.tile_pool(name="win", bufs=6))
        p_wout = ctx.enter_context(tc.tile_pool(name="wout", bufs=4))
        p_axp = ctx.enter_context(tc.tile_pool(name="axp", bufs=4))
        p_sv = ctx.enter_context(tc.tile_pool(name="sv", bufs=3))
        p_diag = ctx.enter_context(tc.tile_pool(name="diag", bufs=4))
        p_res = ctx.enter_context(tc.tile_pool(name="res", bufs=4))
        p_yout = ctx.enter_context(tc.tile_pool(name="yout", bufs=4))
        p_ps = ctx.enter_context(
            tc.tile_pool(name="ps", bufs=8, space=bass.MemorySpace.PSUM))

        ident_f32 = sing.tile([128, 128], f32, name="ident_f32", tag="ident_f32")
        make_identity(nc, ident_f32[:])
        identity = sing.tile([128, 128], f32r, name="identity", tag="identity")
        nc.vector.tensor_copy(identity[:], ident_f32[:])
        magic = sing.tile([128, 1], i32, name="magic", tag="magic")
        nc.vector.memset(magic[:], 0x5F3759DF)

        cw_sb, cbc_sb, hist = {}, {}, {}

        def setup_conv_state():
            for mod in MODS:
                cw_sb[mod] = sing.tile([128, NXC * 4], f32, name=f"cw_{mod}",
                                       tag=f"cw_{mod}")
                nc.sync.dma_start(cw_sb[mod][:], cw_dram[mod][:])
                cbc_sb[mod] = sing.tile([128, NXC], f32, name=f"cbc_{mod}",
                                        tag=f"cbc_{mod}")
                nc.sync.dma_start(cbc_sb[mod][:], cbc_dram[mod][:])
                hist[mod] = sing.tile([128, NXC * 3], f32, name=f"hist_{mod}",
                                      tag=f"hist_{mod}")
                nc.vector.memset(hist[mod][:], 0.0)

        def emit_T(blk):
            l0 = blk * LB
            xTt = {}
            # schedule the load+square chain ~a half block early (priority-only:
            # the PE transposes keep their natural slot so PSUM isn't grabbed)
            boost = 600 if blk > 0 else 0
            for mod in MODS:
                xTt[mod] = p_xT.tile([128, NKD * LB], f32r, name="xT", tag="xT")
                xts, xns, ssums = [], [], []
                with tc.high_priority(offset=boost):
                    for lt in range(NLT):
                        xt = p_xin.tile([128, DIM], f32, name="xin", tag="xin")
                        nc.sync.dma_start(
                            xt[:], x_dram[mod][l0 + lt * 128: l0 + (lt + 1) * 128, :])
                        xnt = p_xn.tile([128, DIM], f32r, name="xn", tag="xn")
                        ssum = p_stat.tile([128, 1], f32, name="ssum", tag="ssum")
                        nc.scalar.activation(xnt[:], xt[:], AF.Square,
                                             accum_out=ssum[:])
                        xts.append(xt)
                        xns.append(xnt)
                        ssums.append(ssum)
                with tc.high_priority(offset=boost):
                    for lt in range(NLT):
                        m = p_stat.tile([128, 1], f32, name="mvar", tag="mvar")
                        nc.vector.tensor_scalar(m[:], ssums[lt][:], 1.0 / DIM, EPS,
                                                AOP.mult, AOP.add)
                        ts = p_stat.tile([128, 1], f32, name="nsh", tag="nsh")
                        nc.vector.tensor_scalar(ts[:].bitcast(i32), m[:].bitcast(i32),
                                                1, None, AOP.logical_shift_right)
                        yv = p_stat.tile([128, 1], f32, name="ny0", tag="ny0")
                        nc.vector.tensor_tensor(yv[:].bitcast(i32), magic[:],
                                                ts[:].bitcast(i32), AOP.subtract)
                        for it in range(3):
                            aa = p_stat.tile([128, 1], f32, name="na", tag="na")
                            nc.vector.tensor_mul(aa[:], yv[:], yv[:])
                            bb = p_stat.tile([128, 1], f32, name="nb", tag="nb")
                            nc.vector.tensor_mul(bb[:], aa[:], m[:])
                            cc = p_stat.tile([128, 1], f32, name="ncc", tag="ncc")
                            nc.vector.tensor_scalar(cc[:], bb[:], -0.5, 1.5,
                                                    AOP.mult, AOP.add)
                            y2 = p_stat.tile([128, 1], f32, name="nyi", tag="nyi")
                            nc.vector.tensor_mul(y2[:], yv[:], cc[:])
                            yv = y2
                        nc.vector.tensor_scalar(xns[lt][:], xts[lt][:], yv[:], None,
                                                AOP.mult)
                for lt in range(NLT):
                    xnt = xns[lt]
                    for g in range(2):
                        pt = p_ps.tile([128, 512], f32, name="ps", tag="ps")
                        for j in range(4):
                            dc = g * 4 + j
                            nc.tensor.matmul(
                                pt[:, j * 128:(j + 1) * 128].bitcast(f32r),
                                lhsT=xnt[:, dc * 128:(dc + 1) * 128],
                                rhs=identity[:],
                                is_transpose=True, skip_group_check=True)
                        dst = xTt[mod].rearrange("p (dc l) -> p dc l", dc=NKD)[
                            :, g * 4:(g + 1) * 4, lt * 128:(lt + 1) * 128]
                        src = pt[:].rearrange("p (j l) -> p j l", j=4)
                        with tc.high_priority(offset=boost):
                            if g == 0:
                                nc.vector.tensor_copy(dst, src)
                            else:
                                nc.scalar.copy(dst, src)

            return xTt

        def prefetch_w(blk):
            tiles = {}
            for mod in MODS:
                for half, m in (("x", 0), ("w", NXC)):
                    wt = p_win.tile([128, DIM], f32r, name="win", tag="win")
                    nc.sync.dma_start(
                        wt[:].rearrange("p (kc m) -> p kc m", kc=NKD),
                        win_dram[mod][m])
                    tiles[(mod, half)] = wt
            return tiles

        def emit_A(blk, xTt, pre=None):
            gat = {}
            for mod in MODS:
                gat[mod] = p_gat.tile([128, NXC * LB], f32r, name="gat", tag="gat")
            for c in range(NXC):
                pp = {}
                for mod in MODS:
                    for half, m in (("x", c), ("w", NXC + c)):
                        if c == 0 and pre is not None:
                            wt = pre[(mod, half)]
                        else:
                            wt = p_win.tile([128, DIM], f32r, name="win", tag="win")
                            nc.sync.dma_start(
                                wt[:].rearrange("p (kc m) -> p kc m", kc=NKD),
                                win_dram[mod][m])
                        ps = p_ps.tile([128, LB], f32, name="ps", tag="ps")
                        for kc in range(NKD):
                            nc.tensor.matmul(
                                ps[:],
                                lhsT=wt[:, kc * 128:(kc + 1) * 128],
                                rhs=xTt[mod][:, kc * LB:(kc + 1) * LB],
                                start=(kc == 0), stop=(kc == NKD - 1))
                        pp[(mod, half)] = ps
                sv = {}
                for mod in MODS:
                    s = p_sv.tile([128, LB], f32, name="sv", tag="sv")
                    nc.scalar.activation(s[:], pp[(mod, "w")][:], AF.Silu)
                    sv[mod] = s
                for mi, (mod, other) in enumerate((("a", "v"), ("v", "a"))):
                    if conv_mode == "pe":
                        cm = "pe"
                    elif conv_mode == "dve":
                        cm = "dve"
                    elif conv_mode == "gps":
                        cm = "gps"
                    elif conv_mode == "dve+gps":
                        cm = "dve" if mi == 0 else "gps"
                    elif conv_mode == "pe+dve":
                        cm = "pe" if mi == 0 else "dve"
                    else:
                        raise ValueError(conv_mode)
                    axp = p_axp.tile([128, LB + 3], f32r, name="axp", tag="axp")
                    nc.vector.tensor_copy(axp[:, 0:3],
                                          hist[mod][:, c * 3:(c + 1) * 3])
                    nc.scalar.copy(axp[:, 3:LB + 3], pp[(mod, "x")][:])
                    nc.vector.tensor_copy(hist[mod][:, c * 3:(c + 1) * 3],
                                          axp[:, LB:LB + 3])
                    if cm == "pe":
                        cp = p_ps.tile([128, LB], f32, name="ps", tag="ps")
                        for t in range(4):
                            dg = p_diag.tile([128, 128], f32r, name="diag", tag="diag")
                            nc.vector.tensor_scalar(
                                dg[:], identity[:],
                                cw_sb[mod][:, c * 4 + t: c * 4 + t + 1],
                                None, AOP.mult)
                            nc.tensor.matmul(cp[:], lhsT=dg[:],
                                             rhs=axp[:, t:t + LB],
                                             start=(t == 0), stop=(t == 3))
                        conv_src = cp
                    else:
                        eng = nc.vector if cm == "dve" else nc.gpsimd
                        acc = p_sv.tile([128, LB], f32, name="convacc", tag="convacc")
                        eng.tensor_scalar(
                            acc[:], axp[:, 0:LB],
                            cw_sb[mod][:, c * 4: c * 4 + 1],
                            cbc_sb[mod][:, c:c + 1], AOP.mult, AOP.add)
                        for t in range(1, 4):
                            acc2 = p_sv.tile([128, LB], f32, name="convacc", tag="convacc")
                            eng.scalar_tensor_tensor(
                                acc2[:], axp[:, t:t + LB],
                                cw_sb[mod][:, c * 4 + t: c * 4 + t + 1],
                                acc[:], AOP.mult, AOP.add)
                            acc = acc2
                        conv_src = acc
                    if cm == "pe":
                        nc.vector.scalar_tensor_tensor(
                            gat[mod][:, c * LB:(c + 1) * LB], conv_src[:],
                            cbc_sb[mod][:, c:c + 1], sv[other][:],
                            AOP.add, AOP.mult)
                    else:
                        nc.vector.tensor_mul(gat[mod][:, c * LB:(c + 1) * LB],
                                             conv_src[:], sv[other][:])

            return gat

        def emit_B(blk, gat):
            l0 = blk * LB
            for mod in MODS:
                yoff = 0 if mod == "a" else L
                for n in range(NN):
                    po = [p_ps.tile([128, 512], f32, name="ps", tag="ps")
                          for _ in range(NLT)]
                    for c2 in range(NXC):
                        w = p_wout.tile([128, 512], f32r, name="wout", tag="wout")
                        nc.sync.dma_start(w[:], wout_dram[mod][c2, n])
                        for mt in range(NLT):
                            nc.tensor.matmul(
                                po[mt][:],
                                lhsT=gat[mod][:, c2 * LB + mt * 128:
                                              c2 * LB + (mt + 1) * 128],
                                rhs=w[:],
                                start=(c2 == 0), stop=(c2 == NXC - 1))
                    for mt in range(NLT):
                        rt = p_res.tile([128, 512], f32, name="res", tag="res")
                        nc.sync.dma_start(
                            rt[:], x_dram[mod][l0 + mt * 128: l0 + (mt + 1) * 128,
                                               n * 512:(n + 1) * 512])
                        yt = p_yout.tile([128, 512], f32, name="yout", tag="yout")
                        nc.vector.tensor_add(yt[:], po[mt][:], rt[:])
                        nc.sync.dma_start(
                            y[yoff + l0 + mt * 128: yoff + l0 + (mt + 1) * 128,
                              n * 512:(n + 1) * 512], yt[:])

        setup_conv_state()
        for rep in range(repeat):
            pre = None
            for blk in range(NB):
                xTt_cur = emit_T(blk)
                gat = emit_A(blk, xTt_cur, pre)
                pre = prefetch_w(blk + 1) if blk + 1 < NB else None
                emit_B(blk, gat)

    nc.finalize()
    return nc


def _get_nc(conv_mode=None, repeat=1):
    key = ("nc", conv_mode or CONV_MODE, repeat)
    if key not in _cache:
        _cache[key] = _build_nc(conv_mode, repeat)
    return _cache[key]


def _prep_weights(inputs):
    f = np.float32
    a_in = np.asarray(inputs["a_in_w"], f) * np.asarray(inputs["a_norm_w"], f)[None, :]
    v_in = np.asarray(inputs["v_in_w"], f) * np.asarray(inputs["v_norm_w"], f)[None, :]

    def pack_in(w):  # [2*INNER, DIM] -> [32, 128, 8, 128]: m-tile x [d%128, d//128, e]
        t = w.T.reshape(NKD, 128, 2 * NXC, 128)   # [kc, p, m, e]
        return np.ascontiguousarray(t.transpose(2, 1, 0, 3))

    def pack_out(w):  # [DIM, INNER] -> [16, 2, 128, 512] (e-chunk, d-tile)
        return np.ascontiguousarray(
            w.T.reshape(NXC, 128, NN, 512).transpose(0, 2, 1, 3))

    def pack_cw(w):  # [INNER, 1, 4] -> [128, 64]
        return np.ascontiguousarray(
            np.asarray(w, f)[:, 0, :].reshape(NXC, 128, 4)
            .transpose(1, 0, 2).reshape(128, NXC * 4))

    return {
        "wina": pack_in(a_in),
        "winv": pack_in(v_in),
        "wouta": pack_out(np.asarray(inputs["a_out_w"], f)),
        "woutv": pack_out(np.asarray(inputs["v_out_w"], f)),
        "cwa": pack_cw(inputs["a_conv_w"]),
        "cwv": pack_cw(inputs["v_conv_w"]),
        "cbca": np.ascontiguousarray(
            np.asarray(inputs["a_conv_b"], f).reshape(NXC, 128).T),
        "cbcv": np.ascontiguousarray(
            np.asarray(inputs["v_conv_b"], f).reshape(NXC, 128).T),
    }


def _make_in_maps(inputs):
    shared = _prep_weights(inputs)
    audio = np.ascontiguousarray(np.asarray(inputs["audio"], np.float32))
    visual = np.ascontiguousarray(np.asarray(inputs["visual"], np.float32))
    in_maps = []
    for b in range(B):
        m = dict(shared)
        m["xa"] = audio[b]
        m["xv"] = visual[b]
        in_maps.append(m)
    return in_maps


def _run(in_maps):
    from concourse.bass_utils import run_bass_kernel_spmd
    nc = _get_nc()
    out = None
    for _attempt in range(3):
        res = run_bass_kernel_spmd(nc, in_maps, core_ids=list(range(NCORES)))
        out = np.stack([res.results[b]["y"] for b in range(B)], axis=0)
        # Rare terminal-side flake: a dropped write leaves a [128, 512] output
        # tile as the donated zero-buffer contents. Legit all-zero tiles are
        # impossible (residual stream is dense gaussian), so detect and retry.
        v = out.reshape(B, (2 * L) // 128, 128, DIM // 512, 512)
        if np.abs(v).max(axis=(2, 4)).min() > 0.0:
            return out
    return out


def kernel(**inputs) -> np.ndarray:
    return _run(_make_in_maps(inputs))



# revision 3
# speedup vs baseline: 1.0411x; 1.0411x over previous
"""AV temporal gated-conv MLP block for Trainium2 (8 NeuronCores, Bass/Tile).

Per-core strategy: pure data parallelism over the batch (B=8 -> 1 batch
element per core, both modalities on the same core since the gating couples
them). No collectives. Per core, loop over 4 l-blocks of 512 tokens:

  T: rms-norm in natural [l, d] layout -- ACT Square with accum_out for the
     square-sums, rsqrt entirely on DVE (bit-trick seed + 3 Newton
     iterations), per-partition scale, and PE-transpose of x_n to [d, l]
     (bf16 transpose mode: 1 cyc/row, 8 tiles packed per PSUM bank).
  A: in_proj as bf16 matmuls (1 cycle/row PE rate) producing [e, l] chunks;
     causal depthwise conv on the x-half as 1 tensor_scalar + 3
     scalar_tensor_tensor fused MACs on VectorE; silu (ACT) on the w-half
     straight out of PSUM; cross-modal gate writes bf16 gated activations.
  B: out_proj bf16 matmuls against SBUF-RESIDENT out_proj weights (loaded
     once, 8MB bf16 for both modalities); residual add reads the retained
     natural-layout x tiles (no DRAM re-read); bf16 DMA out.

The whole datapath is bf16 (tolerance is 2e-2; measured rel-err ~2e-3):
weights, x, xT, gat, y. This halves every weight restream and roughly
3x-cuts per-core HBM traffic vs the f32r version (252MB -> 88MB), which
matters because the 8 cores share chip HBM: the f32r kernel was
aggregate-HBM-bound on hardware (~2.3TB/s demand), this one is PE-bound.

Engine budget per core (cost model): PE ~662us (matmul floor 655us), DMA
~265us (88MB), DVE ~480us, ACT ~160us; PE-bound end-to-end.
"""
import sys

if "/opt/trn_rl_repo" not in sys.path:
    sys.path.insert(0, "/opt/trn_rl_repo")

import numpy as np

DIM = 1024
INNER = 2048
L = 2048
B = 8
NCORES = 8
EPS = 1e-5
LB = 512              # l-block (tokens per block)
NB = L // LB          # 4 blocks
NXC = INNER // 128    # 16 x-half e-chunks per modality
NKD = DIM // 128      # 8 contraction chunks for in_proj
NLT = LB // 128       # 4 l-tiles per block
NN = DIM // 512       # 2 out_proj n-tiles
CONV_MODE = "dve"  # "dve" | "gps" | "dve+gps"

_cache = {}


def _build_nc(conv_mode=None, repeat=1):
    conv_mode = conv_mode or CONV_MODE
    from contextlib import ExitStack

    import concourse.bass as bass
    import concourse.tile as tile
    from concourse import bacc, mybir
    from concourse.masks import make_identity

    dt = mybir.dt
    f32 = dt.float32
    bf16 = dt.bfloat16
    i32 = dt.int32
    AOP = mybir.AluOpType
    AF = mybir.ActivationFunctionType

    nc = bacc.Bacc("TRN2", target_bir_lowering=False, debug=False,
                   num_devices=NCORES)

    x_dram = {
        "a": nc.dram_tensor("xa", [L, DIM], bf16, kind="ExternalInput").ap(),
        "v": nc.dram_tensor("xv", [L, DIM], bf16, kind="ExternalInput").ap(),
    }
    win_dram = {
        "a": nc.dram_tensor("wina", [2 * NXC, 128, NKD, 128], bf16,
                            kind="ExternalInput").ap(),
        "v": nc.dram_tensor("winv", [2 * NXC, 128, NKD, 128], bf16,
                            kind="ExternalInput").ap(),
    }
    wout_dram = {
        "a": nc.dram_tensor("wouta", [128, NXC * NN * 512], bf16,
                            kind="ExternalInput").ap(),
        "v": nc.dram_tensor("woutv", [128, NXC * NN * 512], bf16,
                            kind="ExternalInput").ap(),
    }
    cw_dram = {
        "a": nc.dram_tensor("cwa", [128, NXC * 4], f32, kind="ExternalInput").ap(),
        "v": nc.dram_tensor("cwv", [128, NXC * 4], f32, kind="ExternalInput").ap(),
    }
    cbc_dram = {
        "a": nc.dram_tensor("cbca", [128, NXC], f32, kind="ExternalInput").ap(),
        "v": nc.dram_tensor("cbcv", [128, NXC], f32, kind="ExternalInput").ap(),
    }
    y = nc.dram_tensor("y", [2 * L, DIM], bf16, kind="ExternalOutput").ap()

    MODS = ("a", "v")

    with tile.TileContext(nc) as tc, ExitStack() as ctx:
        sing = ctx.enter_context(tc.tile_pool(name="sing", bufs=1))
        p_xT = ctx.enter_context(tc.tile_pool(name="xT", bufs=2))
        p_gat = ctx.enter_context(tc.tile_pool(name="gat", bufs=2))
        p_xin = ctx.enter_context(tc.tile_pool(name="xin", bufs=12))
        p_xn = ctx.enter_context(tc.tile_pool(name="xn", bufs=5))
        p_stat = ctx.enter_context(tc.tile_pool(name="stat", bufs=4))
        p_win = ctx.enter_context(tc.tile_pool(name="win", bufs=8))
        p_axp = ctx.enter_context(tc.tile_pool(name="axp", bufs=4))
        p_sv = ctx.enter_context(tc.tile_pool(name="sv", bufs=3))
        p_yout = ctx.enter_context(tc.tile_pool(name="yout", bufs=6))
        p_ps = ctx.enter_context(
            tc.tile_pool(name="ps", bufs=8, space=bass.MemorySpace.PSUM))

        ident_f32 = sing.tile([128, 128], f32, name="ident_f32", tag="ident_f32")
        make_identity(nc, ident_f32[:])
        identity = sing.tile([128, 128], bf16, name="identity", tag="identity")
        nc.vector.tensor_copy(identity[:], ident_f32[:])
        magic = sing.tile([128, 1], i32, name="magic", tag="magic")
        nc.vector.memset(magic[:], 0x5F3759DF)

        cw_sb, cbc_sb, hist, wout_sb = {}, {}, {}, {}

        def setup_conv_state():
            for mod in MODS:
                cw_sb[mod] = sing.tile([128, NXC * 4], f32, name=f"cw_{mod}",
                                       tag=f"cw_{mod}")
                nc.sync.dma_start(cw_sb[mod][:], cw_dram[mod][:])
                cbc_sb[mod] = sing.tile([128, NXC], f32, name=f"cbc_{mod}",
                                        tag=f"cbc_{mod}")
                nc.sync.dma_start(cbc_sb[mod][:], cbc_dram[mod][:])
                hist[mod] = sing.tile([128, NXC * 3], f32, name=f"hist_{mod}",
                                      tag=f"hist_{mod}")
                nc.vector.memset(hist[mod][:], 0.0)

        def load_wout():
            for mod in MODS:
                wout_sb[mod] = sing.tile([128, NXC * NN * 512], bf16,
                                         name=f"wout_{mod}", tag=f"wout_{mod}")
                nc.sync.dma_start(wout_sb[mod][:], wout_dram[mod][:])

        def emit_T(blk):
            l0 = blk * LB
            xTt, xts = {}, {}
            # schedule the load+square chain ~a half block early (priority-only:
            # the PE transposes keep their natural slot so PSUM isn't grabbed)
            boost = 600 if blk > 0 else 0
            for mod in MODS:
                xTt[mod] = p_xT.tile([128, NKD * LB], bf16, name="xT", tag="xT")
                xts[mod] = []
                xns, ssums = [], []
                with tc.high_priority(offset=boost):
                    for lt in range(NLT):
                        xt = p_xin.tile([128, DIM], bf16, name="xin", tag="xin")
                        nc.sync.dma_start(
                            xt[:], x_dram[mod][l0 + lt * 128: l0 + (lt + 1) * 128, :])
                        xnt = p_xn.tile([128, DIM], bf16, name="xn", tag="xn")
                        ssum = p_stat.tile([128, 1], f32, name="ssum", tag="ssum")
                        nc.scalar.activation(xnt[:], xt[:], AF.Square,
                                             accum_out=ssum[:])
                        xts[mod].append(xt)
                        xns.append(xnt)
                        ssums.append(ssum)
                with tc.high_priority(offset=boost):
                    for lt in range(NLT):
                        m = p_stat.tile([128, 1], f32, name="mvar", tag="mvar")
                        nc.vector.tensor_scalar(m[:], ssums[lt][:], 1.0 / DIM, EPS,
                                                AOP.mult, AOP.add)
                        ts = p_stat.tile([128, 1], f32, name="nsh", tag="nsh")
                        nc.vector.tensor_scalar(ts[:].bitcast(i32), m[:].bitcast(i32),
                                                1, None, AOP.logical_shift_right)
                        yv = p_stat.tile([128, 1], f32, name="ny0", tag="ny0")
                        nc.vector.tensor_tensor(yv[:].bitcast(i32), magic[:],
                                                ts[:].bitcast(i32), AOP.subtract)
                        for it in range(3):
                            aa = p_stat.tile([128, 1], f32, name="na", tag="na")
                            nc.vector.tensor_mul(aa[:], yv[:], yv[:])
                            bb = p_stat.tile([128, 1], f32, name="nb", tag="nb")
                            nc.vector.tensor_mul(bb[:], aa[:], m[:])
                            cc = p_stat.tile([128, 1], f32, name="ncc", tag="ncc")
                            nc.vector.tensor_scalar(cc[:], bb[:], -0.5, 1.5,
                                                    AOP.mult, AOP.add)
                            y2 = p_stat.tile([128, 1], f32, name="nyi", tag="nyi")
                            nc.vector.tensor_mul(y2[:], yv[:], cc[:])
                            yv = y2
                        nc.vector.tensor_scalar(xns[lt][:], xts[mod][lt][:], yv[:],
                                                None, AOP.mult)
                for lt in range(NLT):
                    xnt = xns[lt]
                    pt = p_ps.tile([128, NKD * 128], bf16, name="ps", tag="ps")
                    for dc in range(NKD):
                        nc.tensor.matmul(
                            pt[:, dc * 128:(dc + 1) * 128],
                            lhsT=xnt[:, dc * 128:(dc + 1) * 128],
                            rhs=identity[:],
                            is_transpose=True, skip_group_check=True)
                    dst = xTt[mod].rearrange("p (dc l) -> p dc l", dc=NKD)[
                        :, :, lt * 128:(lt + 1) * 128]
                    src = pt[:].rearrange("p (j l) -> p j l", j=NKD)
                    with tc.high_priority(offset=boost):
                        if lt % 2 == 0:
                            nc.vector.tensor_copy(dst, src)
                        else:
                            nc.scalar.copy(dst, src)

            return xTt, xts

        def prefetch_w(blk):
            tiles = {}
            for mod in MODS:
                for half, m in (("x", 0), ("w", NXC)):
                    wt = p_win.tile([128, DIM], bf16, name="win", tag="win")
                    nc.sync.dma_start(
                        wt[:].rearrange("p (kc m) -> p kc m", kc=NKD),
                        win_dram[mod][m])
                    tiles[(mod, half)] = wt
            return tiles

        def emit_A(blk, xTt, pre=None):
            gat = {}
            for mod in MODS:
                gat[mod] = p_gat.tile([128, NXC * LB], bf16, name="gat", tag="gat")
            for c in range(NXC):
                pp = {}
                for mod in MODS:
                    for half, m in (("x", c), ("w", NXC + c)):
                        if c == 0 and pre is not None:
                            wt = pre[(mod, half)]
                        else:
                            wt = p_win.tile([128, DIM], bf16, name="win", tag="win")
                            nc.sync.dma_start(
                                wt[:].rearrange("p (kc m) -> p kc m", kc=NKD),
                                win_dram[mod][m])
                        ps = p_ps.tile([128, LB], f32, name="ps", tag="ps")
                        for kc in range(NKD):
                            nc.tensor.matmul(
                                ps[:],
                                lhsT=wt[:, kc * 128:(kc + 1) * 128],
                                rhs=xTt[mod][:, kc * LB:(kc + 1) * LB],
                                start=(kc == 0), stop=(kc == NKD - 1))
                        pp[(mod, half)] = ps
                sv = {}
                for mod in MODS:
                    s = p_sv.tile([128, LB], f32, name="sv", tag="sv")
                    nc.scalar.activation(s[:], pp[(mod, "w")][:], AF.Silu)
                    sv[mod] = s
                for mi, (mod, other) in enumerate((("a", "v"), ("v", "a"))):
                    if conv_mode == "dve":
                        cm = "dve"
                    elif conv_mode == "gps":
                        cm = "gps"
                    elif conv_mode == "dve+gps":
                        cm = "dve" if mi == 0 else "gps"
                    else:
                        raise ValueError(conv_mode)
                    axp = p_axp.tile([128, LB + 3], f32, name="axp", tag="axp")
                    nc.vector.tensor_copy(axp[:, 0:3],
                                          hist[mod][:, c * 3:(c + 1) * 3])
                    nc.scalar.copy(axp[:, 3:LB + 3], pp[(mod, "x")][:])
                    nc.vector.tensor_copy(hist[mod][:, c * 3:(c + 1) * 3],
                                          axp[:, LB:LB + 3])
                    eng = nc.vector if cm == "dve" else nc.gpsimd
                    acc = p_sv.tile([128, LB], f32, name="convacc", tag="convacc")
                    eng.tensor_scalar(
                        acc[:], axp[:, 0:LB],
                        cw_sb[mod][:, c * 4: c * 4 + 1],
                        cbc_sb[mod][:, c:c + 1], AOP.mult, AOP.add)
                    for t in range(1, 4):
                        acc2 = p_sv.tile([128, LB], f32, name="convacc", tag="convacc")
                        eng.scalar_tensor_tensor(
                            acc2[:], axp[:, t:t + LB],
                            cw_sb[mod][:, c * 4 + t: c * 4 + t + 1],
                            acc[:], AOP.mult, AOP.add)
                        acc = acc2
                    nc.vector.tensor_mul(gat[mod][:, c * LB:(c + 1) * LB],
                                         acc[:], sv[other][:])

            return gat

        def emit_B(blk, gat, xts):
            l0 = blk * LB
            for mod in MODS:
                yoff = 0 if mod == "a" else L
                for n in range(NN):
                    po = [p_ps.tile([128, 512], f32, name="ps", tag="ps")
                          for _ in range(NLT)]
                    for c2 in range(NXC):
                        w = wout_sb[mod][:, (c2 * NN + n) * 512:
                                         (c2 * NN + n + 1) * 512]
                        for mt in range(NLT):
                            nc.tensor.matmul(
                                po[mt][:],
                                lhsT=gat[mod][:, c2 * LB + mt * 128:
                                              c2 * LB + (mt + 1) * 128],
                                rhs=w,
                                start=(c2 == 0), stop=(c2 == NXC - 1))
                    for mt in range(NLT):
                        yt = p_yout.tile([128, 512], bf16, name="yout", tag="yout")
                        nc.vector.tensor_add(
                            yt[:], po[mt][:],
                            xts[mod][mt][:, n * 512:(n + 1) * 512])
                        nc.sync.dma_start(
                            y[yoff + l0 + mt * 128: yoff + l0 + (mt + 1) * 128,
                              n * 512:(n + 1) * 512], yt[:])

        setup_conv_state()
        for rep in range(repeat):
            pre = None
            for blk in range(NB):
                xTt_cur, xts_cur = emit_T(blk)
                if rep == 0 and blk == 0:
                    load_wout()
                gat = emit_A(blk, xTt_cur, pre)
                pre = prefetch_w(blk + 1) if blk + 1 < NB else None
                emit_B(blk, gat, xts_cur)

    nc.finalize()
    return nc


def _get_nc(conv_mode=None, repeat=1):
    key = ("nc", conv_mode or CONV_MODE, repeat)
    if key not in _cache:
        _cache[key] = _build_nc(conv_mode, repeat)
    return _cache[key]


def _bf16(a):
    import ml_dtypes
    return np.asarray(a, np.float32).astype(ml_dtypes.bfloat16)


def _prep_weights(inputs):
    f = np.float32
    a_in = np.asarray(inputs["a_in_w"], f) * np.asarray(inputs["a_norm_w"], f)[None, :]
    v_in = np.asarray(inputs["v_in_w"], f) * np.asarray(inputs["v_norm_w"], f)[None, :]

    def pack_in(w):  # [2*INNER, DIM] -> [32, 128, 8, 128]: m-tile x [d%128, d//128, e]
        t = w.T.reshape(NKD, 128, 2 * NXC, 128)   # [kc, p, m, e]
        return _bf16(np.ascontiguousarray(t.transpose(2, 1, 0, 3)))

    def pack_out(w):  # [DIM, INNER] -> [128, NXC*NN*512]: [e%128, (e//128, d//512, d%512)]
        t = np.asarray(w, f).T.reshape(NXC, 128, NN, 512)   # [c2, p, n, d]
        return _bf16(np.ascontiguousarray(
            t.transpose(1, 0, 2, 3).reshape(128, NXC * NN * 512)))

    def pack_cw(w):  # [INNER, 1, 4] -> [128, 64]
        return np.ascontiguousarray(
            np.asarray(w, f)[:, 0, :].reshape(NXC, 128, 4)
            .transpose(1, 0, 2).reshape(128, NXC * 4))

    return {
        "wina": pack_in(a_in),
        "winv": pack_in(v_in),
        "wouta": pack_out(np.asarray(inputs["a_out_w"], f)),
        "woutv": pack_out(np.asarray(inputs["v_out_w"], f)),
        "cwa": pack_cw(inputs["a_conv_w"]),
        "cwv": pack_cw(inputs["v_conv_w"]),
        "cbca": np.ascontiguousarray(
            np.asarray(inputs["a_conv_b"], f).reshape(NXC, 128).T),
        "cbcv": np.ascontiguousarray(
            np.asarray(inputs["v_conv_b"], f).reshape(NXC, 128).T),
    }


def _make_in_maps(inputs):
    shared = _prep_weights(inputs)
    audio = _bf16(inputs["audio"])
    visual = _bf16(inputs["visual"])
    in_maps = []
    for b in range(B):
        m = dict(shared)
        m["xa"] = np.ascontiguousarray(audio[b])
        m["xv"] = np.ascontiguousarray(visual[b])
        in_maps.append(m)
    return in_maps


def _run(in_maps):
    from concourse.bass_utils import run_bass_kernel_spmd
    nc = _get_nc()
    out = None
    for _attempt in range(3):
        res = run_bass_kernel_spmd(nc, in_maps, core_ids=list(range(NCORES)))
        out = np.stack([res.results[b]["y"] for b in range(B)],
                       axis=0).astype(np.float32)
        # Rare terminal-side flake: a dropped write leaves a [128, 512] output
        # tile as the donated zero-buffer contents. Legit all-zero tiles are
        # impossible (residual stream is dense gaussian), so detect and retry.
        v = out.reshape(B, (2 * L) // 128, 128, DIM // 512, 512)
        if np.abs(v).max(axis=(2, 4)).min() > 0.0:
            return out
    return out


def kernel(**inputs) -> np.ndarray:
    return _run(_make_in_maps(inputs))


# revision 34
# speedup vs baseline: 1.1356x; 1.0908x over previous
"""AV temporal gated-conv MLP block for Trainium2 (8 NeuronCores, Bass/Tile).

Per-core strategy: pure data parallelism over the batch (B=8 -> 1 batch
element per core, both modalities on the same core since the gating couples
them). No collectives. Per core, loop over 4 l-blocks of 512 tokens:

  T: rms-norm in natural [l, d] layout -- ACT Square with accum_out for the
     square-sums, rsqrt entirely on DVE (bit-trick seed + 3 Newton
     iterations), per-partition scale, and PE-transpose of x_n to [d, l]
     (bf16 transpose mode: 1 cyc/row, 8 tiles packed per PSUM bank).
  A: in_proj as bf16 matmuls (1 cycle/row PE rate) producing [e, l] chunks;
     causal depthwise conv on the x-half as 1 tensor_scalar + 3
     scalar_tensor_tensor fused MACs on VectorE; silu (ACT) on the w-half
     straight out of PSUM; cross-modal gate writes bf16 gated activations.
  B: out_proj bf16 matmuls against SBUF-RESIDENT out_proj weights (loaded
     once, 8MB bf16 for both modalities); residual add reads the retained
     natural-layout x tiles (no DRAM re-read); bf16 DMA out.

The whole datapath is bf16 (tolerance is 2e-2; measured rel-err ~2e-3):
weights, x, xT, gat, y. This halves every weight restream and roughly
3x-cuts per-core HBM traffic vs the f32r version (252MB -> 88MB), which
matters because the 8 cores share chip HBM: the f32r kernel was
aggregate-HBM-bound on hardware (~2.3TB/s demand), this one is PE-bound.

Engine budget per core (cost model): PE ~662us (matmul floor 655us), DMA
~265us (88MB), DVE ~480us, ACT ~160us; PE-bound end-to-end.
"""
import sys

if "/opt/trn_rl_repo" not in sys.path:
    sys.path.insert(0, "/opt/trn_rl_repo")

import numpy as np

DIM = 1024
INNER = 2048
L = 2048
B = 8
NCORES = 8
EPS = 1e-5
LB = 512              # l-block (tokens per block)
NB = L // LB          # 4 blocks
NXC = INNER // 128    # 16 x-half e-chunks per modality
NKD = DIM // 128      # 8 contraction chunks for in_proj
NLT = LB // 128       # 4 l-tiles per block
NN = DIM // 512       # 2 out_proj n-tiles
CONV_MODE = "dve"  # "dve" | "gps" | "dve+gps"

_cache = {}


def _build_nc(conv_mode=None, repeat=1):
    conv_mode = conv_mode or CONV_MODE
    from contextlib import ExitStack

    import concourse.bass as bass
    import concourse.tile as tile
    from concourse import bacc, mybir
    from concourse.masks import make_identity

    dt = mybir.dt
    f32 = dt.float32
    bf16 = dt.bfloat16
    i32 = dt.int32
    AOP = mybir.AluOpType
    AF = mybir.ActivationFunctionType

    nc = bacc.Bacc("TRN2", target_bir_lowering=False, debug=False,
                   num_devices=NCORES)

    x_dram = {
        "a": nc.dram_tensor("xa", [L, DIM], bf16, kind="ExternalInput").ap(),
        "v": nc.dram_tensor("xv", [L, DIM], bf16, kind="ExternalInput").ap(),
    }
    win_dram = {
        "a": nc.dram_tensor("wina", [2 * NXC, 128, NKD, 128], bf16,
                            kind="ExternalInput").ap(),
        "v": nc.dram_tensor("winv", [2 * NXC, 128, NKD, 128], bf16,
                            kind="ExternalInput").ap(),
    }
    wout_dram = {
        "a": nc.dram_tensor("wouta", [128, NXC * NN * 512], bf16,
                            kind="ExternalInput").ap(),
        "v": nc.dram_tensor("woutv", [128, NXC * NN * 512], bf16,
                            kind="ExternalInput").ap(),
    }
    cw_dram = {
        "a": nc.dram_tensor("cwa", [128, NXC * 4], f32, kind="ExternalInput").ap(),
        "v": nc.dram_tensor("cwv", [128, NXC * 4], f32, kind="ExternalInput").ap(),
    }
    cbc_dram = {
        "a": nc.dram_tensor("cbca", [128, NXC], f32, kind="ExternalInput").ap(),
        "v": nc.dram_tensor("cbcv", [128, NXC], f32, kind="ExternalInput").ap(),
    }
    y = nc.dram_tensor("y", [2 * L, DIM], bf16, kind="ExternalOutput").ap()

    MODS = ("a", "v")

    with tile.TileContext(nc) as tc, ExitStack() as ctx:
        sing = ctx.enter_context(tc.tile_pool(name="sing", bufs=1))
        p_xT = ctx.enter_context(tc.tile_pool(name="xT", bufs=2))
        p_gat = ctx.enter_context(tc.tile_pool(name="gat", bufs=2))
        p_xin = ctx.enter_context(tc.tile_pool(name="xin", bufs=18))
        p_xn = ctx.enter_context(tc.tile_pool(name="xn", bufs=6))
        p_stat = ctx.enter_context(tc.tile_pool(name="stat", bufs=4))
        p_win = ctx.enter_context(tc.tile_pool(name="win", bufs=6))
        p_axp = ctx.enter_context(tc.tile_pool(name="axp", bufs=4))
        p_sv = ctx.enter_context(tc.tile_pool(name="sv", bufs=3))
        p_yout = ctx.enter_context(tc.tile_pool(name="yout", bufs=4))
        p_ps = ctx.enter_context(
            tc.tile_pool(name="ps", bufs=8, space=bass.MemorySpace.PSUM))

        magic = sing.tile([128, NLT], i32, name="magic", tag="magic")
        nc.vector.memset(magic[:], 0x5F3759DF)

        cw_sb, cbc_sb, hist, wout_sb = {}, {}, {}, {}

        def setup_conv_state():
            for mod in MODS:
                cw_sb[mod] = sing.tile([128, NXC * 4], f32, name=f"cw_{mod}",
                                       tag=f"cw_{mod}")
                nc.sync.dma_start(cw_sb[mod][:], cw_dram[mod][:])
                cbc_sb[mod] = sing.tile([128, NXC], f32, name=f"cbc_{mod}",
                                        tag=f"cbc_{mod}")
                nc.sync.dma_start(cbc_sb[mod][:], cbc_dram[mod][:])
                hist[mod] = sing.tile([128, NXC * 3], f32, name=f"hist_{mod}",
                                      tag=f"hist_{mod}")
                nc.vector.memset(hist[mod][:], 0.0)

        def load_wout():
            # split into 4 chunks per modality so no single 4MB DMA blocks a
            # later win-stream load that lands behind it in the same queue
            NSP = 4
            csz = NXC * NN * 512 // NSP
            for mod in MODS:
                wout_sb[mod] = sing.tile([128, NXC * NN * 512], bf16,
                                         name=f"wout_{mod}", tag=f"wout_{mod}")
                for s in range(NSP):
                    nc.sync.dma_start(wout_sb[mod][:, s * csz:(s + 1) * csz],
                                      wout_dram[mod][:, s * csz:(s + 1) * csz])

        def emit_T(blk, first=False, after_mod=None):
            l0 = blk * LB
            xTt, xts = {}, {}
            # schedule the load+square chain ~a half block early (priority-only:
            # the PE transposes keep their natural slot so PSUM isn't grabbed).
            # First block: strictly stagger mod-a chain > a-weights > mod-v
            # chain > v-weights so the scheduler doesn't round-robin the DMAs
            # (mod-a's chain latency gates the very first matmul).
            for mod in MODS:
                boost = 0 if first else 600
                if after_mod is not None and mod != MODS[0]:
                    after_mod(MODS[0])
                xTt[mod] = p_xT.tile([128, NKD * LB], bf16, name="xT", tag="xT")
                xts[mod] = []
                xns = []
                stats = p_stat.tile([128, NLT], f32, name="ssum", tag="ssum")
                with tc.high_priority(offset=boost):
                    for lt in range(NLT):
                        xt = p_xin.tile([128, DIM], bf16, name="xin", tag="xin")
                        nc.sync.dma_start(
                            xt[:], x_dram[mod][l0 + lt * 128: l0 + (lt + 1) * 128, :])
                        xnt = p_xn.tile([128, DIM], bf16, name="xn", tag="xn")
                        nc.scalar.activation(xnt[:], xt[:], AF.Square,
                                             accum_out=stats[:, lt:lt + 1])
                        xts[mod].append(xt)
                        xns.append(xnt)
                with tc.high_priority(offset=boost):
                    # rsqrt of all NLT square-sums in ONE batched [128, NLT]
                    # Newton chain (bit-trick seed + 3 iterations): 4x fewer
                    # serial DVE ops than per-l-tile chains -- this chain's
                    # latency paces xT availability and hence PE start
                    m = p_stat.tile([128, NLT], f32, name="mvar", tag="mvar")
                    nc.vector.tensor_scalar(m[:], stats[:], 1.0 / DIM, EPS,
                                            AOP.mult, AOP.add)
                    ts = p_stat.tile([128, NLT], f32, name="nsh", tag="nsh")
                    nc.vector.tensor_scalar(ts[:].bitcast(i32), m[:].bitcast(i32),
                                            1, None, AOP.logical_shift_right)
                    yv = p_stat.tile([128, NLT], f32, name="ny0", tag="ny0")
                    nc.vector.tensor_tensor(yv[:].bitcast(i32), magic[:],
                                            ts[:].bitcast(i32), AOP.subtract)
                    mh = p_stat.tile([128, NLT], f32, name="nmh", tag="nmh")
                    nc.vector.tensor_scalar(mh[:], m[:], -0.5, None, AOP.mult)
                    for it in range(3):
                        aa = p_stat.tile([128, NLT], f32, name="na", tag="na")
                        nc.vector.tensor_mul(aa[:], yv[:], yv[:])
                        cc = p_stat.tile([128, NLT], f32, name="ncc", tag="ncc")
                        nc.vector.tensor_tensor(cc[:], aa[:], mh[:], AOP.mult)
                        nc.vector.tensor_scalar(cc[:], cc[:], 1.0, 1.5,
                                                AOP.mult, AOP.add)
                        y2 = p_stat.tile([128, NLT], f32, name="nyi", tag="nyi")
                        nc.vector.tensor_mul(y2[:], yv[:], cc[:])
                        yv = y2
                    for lt in range(NLT):
                        nc.vector.tensor_scalar(xns[lt][:], xts[mod][lt][:],
                                                yv[:, lt:lt + 1], None, AOP.mult)
                with tc.high_priority(offset=boost):
                    for lt in range(NLT):
                        # DMA XBAR transpose: xn [128(l), 1024(d)] -> xT slab
                        # [128(d%128), 8(d//128), 128(l)] in one instruction
                        # (64 16x128 xbar tiles, ~0.9us) -- zero PE involvement
                        dst = xTt[mod].rearrange("p (dc l) -> p dc l", dc=NKD)[
                            :, :, lt * 128:(lt + 1) * 128]
                        nc.sync.dma_start(dst, xns[lt][:], transpose=True)
            if after_mod is not None:
                after_mod(MODS[1])

            return xTt, xts

        def prefetch_w(blk):
            tiles = {}
            for mod in MODS:
                for half, m in (("x", 0), ("w", NXC)):
                    wt = p_win.tile([128, DIM], bf16, name="win", tag="win")
                    nc.sync.dma_start(
                        wt[:].rearrange("p (kc m) -> p kc m", kc=NKD),
                        win_dram[mod][m])
                    tiles[(mod, half)] = wt
            return tiles

        def emit_A(blk, xTt, pre=None):
            gat = {}
            for mod in MODS:
                gat[mod] = p_gat.tile([128, NXC * LB], bf16, name="gat", tag="gat")
            for c in range(NXC):
                pp = {}
                for mod in MODS:
                    for half, m in (("x", c), ("w", NXC + c)):
                        if c == 0 and pre is not None:
                            wt = pre[(mod, half)]
                        else:
                            wt = p_win.tile([128, DIM], bf16, name="win", tag="win")
                            nc.sync.dma_start(
                                wt[:].rearrange("p (kc m) -> p kc m", kc=NKD),
                                win_dram[mod][m])
                        ps = p_ps.tile([128, LB], f32, name="ps", tag="ps")
                        for kc in range(NKD):
                            nc.tensor.matmul(
                                ps[:],
                                lhsT=wt[:, kc * 128:(kc + 1) * 128],
                                rhs=xTt[mod][:, kc * LB:(kc + 1) * LB],
                                start=(kc == 0), stop=(kc == NKD - 1))
                        pp[(mod, half)] = ps
                sv = {}
                for mod in MODS:
                    s = p_sv.tile([128, LB], f32, name="sv", tag="sv")
                    nc.scalar.activation(s[:], pp[(mod, "w")][:], AF.Silu)
                    sv[mod] = s
                for mi, (mod, other) in enumerate((("a", "v"), ("v", "a"))):
                    if conv_mode == "dve":
                        cm = "dve"
                    elif conv_mode == "gps":
                        cm = "gps"
                    elif conv_mode == "dve+gps":
                        cm = "dve" if mi == 0 else "gps"
                    else:
                        raise ValueError(conv_mode)
                    axp = p_axp.tile([128, LB + 3], f32, name="axp", tag="axp")
                    nc.vector.tensor_copy(axp[:, 0:3],
                                          hist[mod][:, c * 3:(c + 1) * 3])
                    nc.scalar.copy(axp[:, 3:LB + 3], pp[(mod, "x")][:])
                    nc.vector.tensor_copy(hist[mod][:, c * 3:(c + 1) * 3],
                                          axp[:, LB:LB + 3])
                    eng = nc.vector if cm == "dve" else nc.gpsimd
                    acc = p_sv.tile([128, LB], f32, name="convacc", tag="convacc")
                    eng.tensor_scalar(
                        acc[:], axp[:, 0:LB],
                        cw_sb[mod][:, c * 4: c * 4 + 1],
                        cbc_sb[mod][:, c:c + 1], AOP.mult, AOP.add)
                    for t in range(1, 4):
                        acc2 = p_sv.tile([128, LB], f32, name="convacc", tag="convacc")
                        eng.scalar_tensor_tensor(
                            acc2[:], axp[:, t:t + LB],
                            cw_sb[mod][:, c * 4 + t: c * 4 + t + 1],
                            acc[:], AOP.mult, AOP.add)
                        acc = acc2
                    nc.vector.tensor_mul(gat[mod][:, c * LB:(c + 1) * LB],
                                         acc[:], sv[other][:])

            return gat

        def emit_B(blk, gat, xts):
            l0 = blk * LB
            for mod in MODS:
                yoff = 0 if mod == "a" else L
                for n in range(NN):
                    po = [p_ps.tile([128, 512], f32, name="ps", tag="ps")
                          for _ in range(NLT)]
                    for c2 in range(NXC):
                        w = wout_sb[mod][:, (c2 * NN + n) * 512:
                                         (c2 * NN + n + 1) * 512]
                        for mt in range(NLT):
                            nc.tensor.matmul(
                                po[mt][:],
                                lhsT=gat[mod][:, c2 * LB + mt * 128:
                                              c2 * LB + (mt + 1) * 128],
                                rhs=w,
                                start=(c2 == 0), stop=(c2 == NXC - 1))
                    for mt in range(NLT):
                        yt = p_yout.tile([128, 512], bf16, name="yout", tag="yout")
                        nc.vector.tensor_add(
                            yt[:], po[mt][:],
                            xts[mod][mt][:, n * 512:(n + 1) * 512])
                        nc.sync.dma_start(
                            y[yoff + l0 + mt * 128: yoff + l0 + (mt + 1) * 128,
                              n * 512:(n + 1) * 512], yt[:])

        # Flattened (rep, blk) sequence. Per step: A(blk) is emitted, then
        # T(blk+1) and the next weight prefetch, then B(blk). The c0 weight
        # pair is prefetched before T(0) so the first matmul waits only on
        # the first xT transpose chain; conv state DMAs queue after x loads.
        nsteps = repeat * NB
        # Startup-ordered emission: mod-a's T chain first, then its c0 weight
        # tiles (so the first in_proj matmuls start as soon as xT_a lands),
        # then mod-v's chain + weights, and conv state (needed ~30us in) last.
        pre = {}

        def _prefetch_mod(mod):
            for half, m in (("x", 0), ("w", NXC)):
                wt = p_win.tile([128, DIM], bf16, name="win", tag="win")
                nc.sync.dma_start(
                    wt[:].rearrange("p (kc m) -> p kc m", kc=NKD),
                    win_dram[mod][m])
                pre[(mod, half)] = wt

        cur = emit_T(0, first=True, after_mod=_prefetch_mod)
        setup_conv_state()
        for step in range(nsteps):
            blk = step % NB
            gat = emit_A(blk, cur[0], pre)
            if step == 0:
                load_wout()
            if step + 1 < nsteps:
                pre = prefetch_w((step + 1) % NB)
                nxt = emit_T((step + 1) % NB)
            emit_B(blk, gat, cur[1])
            if step + 1 < nsteps:
                cur = nxt

    nc.finalize()
    return nc


def _get_nc(conv_mode=None, repeat=1):
    key = ("nc", conv_mode or CONV_MODE, repeat)
    if key not in _cache:
        _cache[key] = _build_nc(conv_mode, repeat)
    return _cache[key]


def _bf16(a):
    import ml_dtypes
    return np.asarray(a, np.float32).astype(ml_dtypes.bfloat16)


def _prep_weights(inputs):
    f = np.float32
    a_in = np.asarray(inputs["a_in_w"], f) * np.asarray(inputs["a_norm_w"], f)[None, :]
    v_in = np.asarray(inputs["v_in_w"], f) * np.asarray(inputs["v_norm_w"], f)[None, :]

    def pack_in(w):  # [2*INNER, DIM] -> [32, 128, 8, 128]: m-tile x [d%128, d//128, e]
        t = w.T.reshape(NKD, 128, 2 * NXC, 128)   # [kc, p, m, e]
        return _bf16(np.ascontiguousarray(t.transpose(2, 1, 0, 3)))

    def pack_out(w):  # [DIM, INNER] -> [128, NXC*NN*512]: [e%128, (e//128, d//512, d%512)]
        t = np.asarray(w, f).T.reshape(NXC, 128, NN, 512)   # [c2, p, n, d]
        return _bf16(np.ascontiguousarray(
            t.transpose(1, 0, 2, 3).reshape(128, NXC * NN * 512)))

    def pack_cw(w):  # [INNER, 1, 4] -> [128, 64]
        return np.ascontiguousarray(
            np.asarray(w, f)[:, 0, :].reshape(NXC, 128, 4)
            .transpose(1, 0, 2).reshape(128, NXC * 4))

    return {
        "wina": pack_in(a_in),
        "winv": pack_in(v_in),
        "wouta": pack_out(np.asarray(inputs["a_out_w"], f)),
        "woutv": pack_out(np.asarray(inputs["v_out_w"], f)),
        "cwa": pack_cw(inputs["a_conv_w"]),
        "cwv": pack_cw(inputs["v_conv_w"]),
        "cbca": np.ascontiguousarray(
            np.asarray(inputs["a_conv_b"], f).reshape(NXC, 128).T),
        "cbcv": np.ascontiguousarray(
            np.asarray(inputs["v_conv_b"], f).reshape(NXC, 128).T),
    }


def _make_in_maps(inputs):
    shared = _prep_weights(inputs)
    audio = _bf16(inputs["audio"])
    visual = _bf16(inputs["visual"])
    in_maps = []
    for b in range(B):
        m = dict(shared)
        m["xa"] = np.ascontiguousarray(audio[b])
        m["xv"] = np.ascontiguousarray(visual[b])
        in_maps.append(m)
    return in_maps


def _run(in_maps):
    from concourse.bass_utils import run_bass_kernel_spmd
    nc = _get_nc()
    out = None
    for _attempt in range(3):
        res = run_bass_kernel_spmd(nc, in_maps, core_ids=list(range(NCORES)))
        out = np.stack([res.results[b]["y"] for b in range(B)],
                       axis=0).astype(np.float32)
        # Rare terminal-side flake: a dropped write leaves a [128, 512] output
        # tile as the donated zero-buffer contents. Legit all-zero tiles are
        # impossible (residual stream is dense gaussian), so detect and retry.
        v = out.reshape(B, (2 * L) // 128, 128, DIM // 512, 512)
        if np.abs(v).max(axis=(2, 4)).min() > 0.0:
            return out
    return out


def kernel(**inputs) -> np.ndarray:
    return _run(_make_in_maps(inputs))


# revision 38
# speedup vs baseline: 1.1806x; 1.0397x over previous
"""AV temporal gated-conv MLP block for Trainium2 (8 NeuronCores, Bass/Tile).

Per-core strategy: pure data parallelism over the batch (B=8 -> 1 batch
element per core, both modalities on the same core since the gating couples
them). No collectives. Per core, loop over 4 l-blocks of 512 tokens:

  T: rms-norm in natural [l, d] layout -- ACT Square with accum_out for the
     square-sums, rsqrt entirely on DVE (bit-trick seed + 3 Newton
     iterations), per-partition scale, and PE-transpose of x_n to [d, l]
     (bf16 transpose mode: 1 cyc/row, 8 tiles packed per PSUM bank).
  A: in_proj as bf16 matmuls (1 cycle/row PE rate) producing [e, l] chunks;
     causal depthwise conv on the x-half as 1 tensor_scalar + 3
     scalar_tensor_tensor fused MACs on VectorE; silu (ACT) on the w-half
     straight out of PSUM; cross-modal gate writes bf16 gated activations.
  B: out_proj bf16 matmuls against SBUF-RESIDENT out_proj weights (loaded
     once, 8MB bf16 for both modalities); residual add reads the retained
     natural-layout x tiles (no DRAM re-read); bf16 DMA out.

The whole datapath is bf16 (tolerance is 2e-2; measured rel-err ~2e-3):
weights, x, xT, gat, y. This halves every weight restream and roughly
3x-cuts per-core HBM traffic vs the f32r version (252MB -> 88MB), which
matters because the 8 cores share chip HBM: the f32r kernel was
aggregate-HBM-bound on hardware (~2.3TB/s demand), this one is PE-bound.

Engine budget per core (cost model): PE ~662us (matmul floor 655us), DMA
~265us (88MB), DVE ~480us, ACT ~160us; PE-bound end-to-end.
"""
import sys

if "/opt/trn_rl_repo" not in sys.path:
    sys.path.insert(0, "/opt/trn_rl_repo")

import numpy as np

DIM = 1024
INNER = 2048
L = 2048
B = 8
NCORES = 8
EPS = 1e-5
LB = 512              # l-block (tokens per block)
NB = L // LB          # 4 blocks
NXC = INNER // 128    # 16 x-half e-chunks per modality
NKD = DIM // 128      # 8 contraction chunks for in_proj
NLT = LB // 128       # 4 l-tiles per block
NN = DIM // 512       # 2 out_proj n-tiles
CONV_MODE = "dve"  # "dve" | "gps" | "dve+gps"  (gps fails in neuronxcc)

_cache = {}


def _build_nc(conv_mode=None, repeat=1):
    conv_mode = conv_mode or CONV_MODE
    from contextlib import ExitStack

    import concourse.bass as bass
    import concourse.tile as tile
    from concourse import bacc, mybir
    from concourse.masks import make_identity

    dt = mybir.dt
    f32 = dt.float32
    bf16 = dt.bfloat16
    i32 = dt.int32
    AOP = mybir.AluOpType
    AF = mybir.ActivationFunctionType

    nc = bacc.Bacc("TRN2", target_bir_lowering=False, debug=False,
                   num_devices=NCORES)

    x_dram = {
        "a": nc.dram_tensor("xa", [L, DIM], bf16, kind="ExternalInput").ap(),
        "v": nc.dram_tensor("xv", [L, DIM], bf16, kind="ExternalInput").ap(),
    }
    win_dram = {
        "a": nc.dram_tensor("wina", [2 * NXC, 128, NKD, 128], bf16,
                            kind="ExternalInput").ap(),
        "v": nc.dram_tensor("winv", [2 * NXC, 128, NKD, 128], bf16,
                            kind="ExternalInput").ap(),
    }
    wout_dram = {
        "a": nc.dram_tensor("wouta", [128, NXC * NN * 512], bf16,
                            kind="ExternalInput").ap(),
        "v": nc.dram_tensor("woutv", [128, NXC * NN * 512], bf16,
                            kind="ExternalInput").ap(),
    }
    cw_dram = {
        "a": nc.dram_tensor("cwa", [128, NXC * 4], f32, kind="ExternalInput").ap(),
        "v": nc.dram_tensor("cwv", [128, NXC * 4], f32, kind="ExternalInput").ap(),
    }
    cbc_dram = {
        "a": nc.dram_tensor("cbca", [128, NXC], f32, kind="ExternalInput").ap(),
        "v": nc.dram_tensor("cbcv", [128, NXC], f32, kind="ExternalInput").ap(),
    }
    y = nc.dram_tensor("y", [2 * L, DIM], bf16, kind="ExternalOutput").ap()

    MODS = ("a", "v")

    with tile.TileContext(nc) as tc, ExitStack() as ctx:
        sing = ctx.enter_context(tc.tile_pool(name="sing", bufs=1))
        p_xT = ctx.enter_context(tc.tile_pool(name="xT", bufs=2))
        p_gat = ctx.enter_context(tc.tile_pool(name="gat", bufs=2))
        p_xin = ctx.enter_context(tc.tile_pool(name="xin", bufs=18))
        p_xn = ctx.enter_context(tc.tile_pool(name="xn", bufs=6))
        p_stat = ctx.enter_context(tc.tile_pool(name="stat", bufs=4))
        p_win = ctx.enter_context(tc.tile_pool(name="win", bufs=6))
        p_axp = ctx.enter_context(tc.tile_pool(name="axp", bufs=4))
        p_sv = ctx.enter_context(tc.tile_pool(name="sv", bufs=3))
        p_yout = ctx.enter_context(tc.tile_pool(name="yout", bufs=4))
        p_ps = ctx.enter_context(
            tc.tile_pool(name="ps", bufs=8, space=bass.MemorySpace.PSUM))

        magic = sing.tile([128, NLT], i32, name="magic", tag="magic")
        nc.vector.memset(magic[:], 0x5F3759DF)

        cw_sb, cbc_sb, hist, wout_sb = {}, {}, {}, {}

        def setup_conv_state():
            for mod in MODS:
                cw_sb[mod] = sing.tile([128, NXC * 4], f32, name=f"cw_{mod}",
                                       tag=f"cw_{mod}")
                nc.sync.dma_start(cw_sb[mod][:], cw_dram[mod][:])
                cbc_sb[mod] = sing.tile([128, NXC], f32, name=f"cbc_{mod}",
                                        tag=f"cbc_{mod}")
                nc.sync.dma_start(cbc_sb[mod][:], cbc_dram[mod][:])
                hist[mod] = sing.tile([128, NXC * 3], bf16, name=f"hist_{mod}",
                                      tag=f"hist_{mod}")
                nc.vector.memset(hist[mod][:], 0.0)

        def load_wout():
            # split into 4 chunks per modality so no single 4MB DMA blocks a
            # later win-stream load that lands behind it in the same queue
            NSP = 4
            csz = NXC * NN * 512 // NSP
            for mod in MODS:
                wout_sb[mod] = sing.tile([128, NXC * NN * 512], bf16,
                                         name=f"wout_{mod}", tag=f"wout_{mod}")
                for s in range(NSP):
                    nc.sync.dma_start(wout_sb[mod][:, s * csz:(s + 1) * csz],
                                      wout_dram[mod][:, s * csz:(s + 1) * csz])

        def emit_T(blk, first=False, after_mod=None):
            l0 = blk * LB
            xTt, xts = {}, {}
            # schedule the load+square chain ~a half block early (priority-only:
            # the PE transposes keep their natural slot so PSUM isn't grabbed).
            # First block: strictly stagger mod-a chain > a-weights > mod-v
            # chain > v-weights so the scheduler doesn't round-robin the DMAs
            # (mod-a's chain latency gates the very first matmul).
            for mod in MODS:
                boost = 0 if first else 600
                if after_mod is not None and mod != MODS[0]:
                    after_mod(MODS[0])
                xTt[mod] = p_xT.tile([128, NKD * LB], bf16, name="xT", tag="xT")
                xts[mod] = []
                xns = []
                stats = p_stat.tile([128, NLT], f32, name="ssum", tag="ssum")
                with tc.high_priority(offset=boost):
                    for lt in range(NLT):
                        xt = p_xin.tile([128, DIM], bf16, name="xin", tag="xin")
                        nc.sync.dma_start(
                            xt[:], x_dram[mod][l0 + lt * 128: l0 + (lt + 1) * 128, :])
                        xnt = p_xn.tile([128, DIM], bf16, name="xn", tag="xn")
                        nc.scalar.activation(xnt[:], xt[:], AF.Square,
                                             accum_out=stats[:, lt:lt + 1])
                        xts[mod].append(xt)
                        xns.append(xnt)
                with tc.high_priority(offset=boost):
                    # rsqrt of all NLT square-sums in ONE batched [128, NLT]
                    # Newton chain (bit-trick seed + 3 iterations): 4x fewer
                    # serial DVE ops than per-l-tile chains -- this chain's
                    # latency paces xT availability and hence PE start
                    m = p_stat.tile([128, NLT], f32, name="mvar", tag="mvar")
                    nc.vector.tensor_scalar(m[:], stats[:], 1.0 / DIM, EPS,
                                            AOP.mult, AOP.add)
                    ts = p_stat.tile([128, NLT], f32, name="nsh", tag="nsh")
                    nc.vector.tensor_scalar(ts[:].bitcast(i32), m[:].bitcast(i32),
                                            1, None, AOP.logical_shift_right)
                    yv = p_stat.tile([128, NLT], f32, name="ny0", tag="ny0")
                    nc.vector.tensor_tensor(yv[:].bitcast(i32), magic[:],
                                            ts[:].bitcast(i32), AOP.subtract)
                    mh = p_stat.tile([128, NLT], f32, name="nmh", tag="nmh")
                    nc.vector.tensor_scalar(mh[:], m[:], -0.5, None, AOP.mult)
                    for it in range(3):
                        aa = p_stat.tile([128, NLT], f32, name="na", tag="na")
                        nc.vector.tensor_mul(aa[:], yv[:], yv[:])
                        cc = p_stat.tile([128, NLT], f32, name="ncc", tag="ncc")
                        nc.vector.tensor_tensor(cc[:], aa[:], mh[:], AOP.mult)
                        nc.vector.tensor_scalar(cc[:], cc[:], 1.0, 1.5,
                                                AOP.mult, AOP.add)
                        y2 = p_stat.tile([128, NLT], f32, name="nyi", tag="nyi")
                        nc.vector.tensor_mul(y2[:], yv[:], cc[:])
                        yv = y2
                    for lt in range(NLT):
                        nc.vector.tensor_scalar(xns[lt][:], xts[mod][lt][:],
                                                yv[:, lt:lt + 1], None, AOP.mult)
                with tc.high_priority(offset=boost):
                    for lt in range(NLT):
                        # DMA XBAR transpose: xn [128(l), 1024(d)] -> xT slab
                        # [128(d%128), 8(d//128), 128(l)] in one instruction
                        # (64 16x128 xbar tiles, ~0.9us) -- zero PE involvement
                        dst = xTt[mod].rearrange("p (dc l) -> p dc l", dc=NKD)[
                            :, :, lt * 128:(lt + 1) * 128]
                        nc.sync.dma_start(dst, xns[lt][:], transpose=True)
            if after_mod is not None:
                after_mod(MODS[1])

            return xTt, xts

        def prefetch_w(blk):
            tiles = {}
            for mod in MODS:
                for half, m in (("x", 0), ("w", NXC)):
                    wt = p_win.tile([128, DIM], bf16, name="win", tag="win")
                    nc.sync.dma_start(
                        wt[:].rearrange("p (kc m) -> p kc m", kc=NKD),
                        win_dram[mod][m])
                    tiles[(mod, half)] = wt
            return tiles

        def emit_A(blk, xTt, pre=None):
            gat = {}
            for mod in MODS:
                gat[mod] = p_gat.tile([128, NXC * LB], bf16, name="gat", tag="gat")
            for c in range(NXC):
                pp = {}
                for mod in MODS:
                    for half, m in (("x", c), ("w", NXC + c)):
                        if c == 0 and pre is not None:
                            wt = pre[(mod, half)]
                        else:
                            wt = p_win.tile([128, DIM], bf16, name="win", tag="win")
                            nc.sync.dma_start(
                                wt[:].rearrange("p (kc m) -> p kc m", kc=NKD),
                                win_dram[mod][m])
                        ps = p_ps.tile([128, LB], f32, name="ps", tag="ps")
                        for kc in range(NKD):
                            nc.tensor.matmul(
                                ps[:],
                                lhsT=wt[:, kc * 128:(kc + 1) * 128],
                                rhs=xTt[mod][:, kc * LB:(kc + 1) * LB],
                                start=(kc == 0), stop=(kc == NKD - 1))
                        pp[(mod, half)] = ps
                sv = {}
                for mod in MODS:
                    # silu straight out of PSUM, bf16 result (2x DVE gate rate)
                    s = p_sv.tile([128, LB], bf16, name="sv", tag="sv")
                    nc.scalar.activation(s[:], pp[(mod, "w")][:], AF.Silu)
                    sv[mod] = s
                for mi, (mod, other) in enumerate((("a", "v"), ("v", "a"))):
                    if conv_mode == "dve":
                        cm = "dve"
                    elif conv_mode == "gps":
                        cm = "gps"
                    elif conv_mode == "dve+gps":
                        cm = "dve" if mi == 0 else "gps"
                    else:
                        raise ValueError(conv_mode)
                    # bf16 conv: halves DVE MAC time; the tiny 3-col history
                    # prepend/save copies run on ACT, not DVE -- per-instr
                    # overhead on HW (~+124ns) makes 3-col DVE ops pure waste
                    axp = p_axp.tile([128, LB + 3], bf16, name="axp", tag="axp")
                    nc.scalar.copy(axp[:, 0:3],
                                   hist[mod][:, c * 3:(c + 1) * 3])
                    nc.scalar.copy(axp[:, 3:LB + 3], pp[(mod, "x")][:])
                    nc.scalar.copy(hist[mod][:, c * 3:(c + 1) * 3],
                                   axp[:, LB:LB + 3])
                    eng = nc.vector if cm == "dve" else nc.gpsimd
                    acc = p_sv.tile([128, LB], bf16, name="convacc", tag="convacc")
                    eng.tensor_scalar(
                        acc[:], axp[:, 0:LB],
                        cw_sb[mod][:, c * 4: c * 4 + 1],
                        cbc_sb[mod][:, c:c + 1], AOP.mult, AOP.add)
                    for t in range(1, 4):
                        acc2 = p_sv.tile([128, LB], bf16, name="convacc", tag="convacc")
                        eng.scalar_tensor_tensor(
                            acc2[:], axp[:, t:t + LB],
                            cw_sb[mod][:, c * 4 + t: c * 4 + t + 1],
                            acc[:], AOP.mult, AOP.add)
                        acc = acc2
                    nc.vector.tensor_mul(gat[mod][:, c * LB:(c + 1) * LB],
                                         acc[:], sv[other][:])

            return gat

        def emit_B(blk, gat, xts):
            l0 = blk * LB
            for mod in MODS:
                yoff = 0 if mod == "a" else L
                for n in range(NN):
                    po = [p_ps.tile([128, 512], f32, name="ps", tag="ps")
                          for _ in range(NLT)]
                    for c2 in range(NXC):
                        w = wout_sb[mod][:, (c2 * NN + n) * 512:
                                         (c2 * NN + n + 1) * 512]
                        for mt in range(NLT):
                            nc.tensor.matmul(
                                po[mt][:],
                                lhsT=gat[mod][:, c2 * LB + mt * 128:
                                              c2 * LB + (mt + 1) * 128],
                                rhs=w,
                                start=(c2 == 0), stop=(c2 == NXC - 1))
                    for mt in range(NLT):
                        yt = p_yout.tile([128, 512], bf16, name="yout", tag="yout")
                        nc.vector.tensor_add(
                            yt[:], po[mt][:],
                            xts[mod][mt][:, n * 512:(n + 1) * 512])
                        nc.sync.dma_start(
                            y[yoff + l0 + mt * 128: yoff + l0 + (mt + 1) * 128,
                              n * 512:(n + 1) * 512], yt[:])

        # Flattened (rep, blk) sequence. Per step: A(blk) is emitted, then
        # T(blk+1) and the next weight prefetch, then B(blk). The c0 weight
        # pair is prefetched before T(0) so the first matmul waits only on
        # the first xT transpose chain; conv state DMAs queue after x loads.
        nsteps = repeat * NB
        # Startup-ordered emission: mod-a's T chain first, then its c0 weight
        # tiles (so the first in_proj matmuls start as soon as xT_a lands),
        # then mod-v's chain + weights, and conv state (needed ~30us in) last.
        pre = {}

        def _prefetch_mod(mod):
            for half, m in (("x", 0), ("w", NXC)):
                wt = p_win.tile([128, DIM], bf16, name="win", tag="win")
                nc.sync.dma_start(
                    wt[:].rearrange("p (kc m) -> p kc m", kc=NKD),
                    win_dram[mod][m])
                pre[(mod, half)] = wt

        cur = emit_T(0, first=True, after_mod=_prefetch_mod)
        setup_conv_state()
        for step in range(nsteps):
            blk = step % NB
            gat = emit_A(blk, cur[0], pre)
            if step == 0:
                load_wout()
            if step + 1 < nsteps:
                pre = prefetch_w((step + 1) % NB)
                nxt = emit_T((step + 1) % NB)
            emit_B(blk, gat, cur[1])
            if step + 1 < nsteps:
                cur = nxt

    nc.finalize()
    return nc


def _get_nc(conv_mode=None, repeat=1):
    key = ("nc", conv_mode or CONV_MODE, repeat)
    if key not in _cache:
        _cache[key] = _build_nc(conv_mode, repeat)
    return _cache[key]


def _bf16(a):
    import ml_dtypes
    return np.asarray(a, np.float32).astype(ml_dtypes.bfloat16)


def _prep_weights(inputs):
    f = np.float32
    a_in = np.asarray(inputs["a_in_w"], f) * np.asarray(inputs["a_norm_w"], f)[None, :]
    v_in = np.asarray(inputs["v_in_w"], f) * np.asarray(inputs["v_norm_w"], f)[None, :]

    def pack_in(w):  # [2*INNER, DIM] -> [32, 128, 8, 128]: m-tile x [d%128, d//128, e]
        t = w.T.reshape(NKD, 128, 2 * NXC, 128)   # [kc, p, m, e]
        return _bf16(np.ascontiguousarray(t.transpose(2, 1, 0, 3)))

    def pack_out(w):  # [DIM, INNER] -> [128, NXC*NN*512]: [e%128, (e//128, d//512, d%512)]
        t = np.asarray(w, f).T.reshape(NXC, 128, NN, 512)   # [c2, p, n, d]
        return _bf16(np.ascontiguousarray(
            t.transpose(1, 0, 2, 3).reshape(128, NXC * NN * 512)))

    def pack_cw(w):  # [INNER, 1, 4] -> [128, 64]
        return np.ascontiguousarray(
            np.asarray(w, f)[:, 0, :].reshape(NXC, 128, 4)
            .transpose(1, 0, 2).reshape(128, NXC * 4))

    return {
        "wina": pack_in(a_in),
        "winv": pack_in(v_in),
        "wouta": pack_out(np.asarray(inputs["a_out_w"], f)),
        "woutv": pack_out(np.asarray(inputs["v_out_w"], f)),
        "cwa": pack_cw(inputs["a_conv_w"]),
        "cwv": pack_cw(inputs["v_conv_w"]),
        "cbca": np.ascontiguousarray(
            np.asarray(inputs["a_conv_b"], f).reshape(NXC, 128).T),
        "cbcv": np.ascontiguousarray(
            np.asarray(inputs["v_conv_b"], f).reshape(NXC, 128).T),
    }


def _make_in_maps(inputs):
    shared = _prep_weights(inputs)
    audio = _bf16(inputs["audio"])
    visual = _bf16(inputs["visual"])
    in_maps = []
    for b in range(B):
        m = dict(shared)
        m["xa"] = np.ascontiguousarray(audio[b])
        m["xv"] = np.ascontiguousarray(visual[b])
        in_maps.append(m)
    return in_maps


def _run(in_maps):
    from concourse.bass_utils import run_bass_kernel_spmd
    nc = _get_nc()
    out = None
    for _attempt in range(3):
        res = run_bass_kernel_spmd(nc, in_maps, core_ids=list(range(NCORES)))
        out = np.stack([res.results[b]["y"] for b in range(B)],
                       axis=0).astype(np.float32)
        # Rare terminal-side flake: a dropped write leaves a [128, 512] output
        # tile as the donated zero-buffer contents. Legit all-zero tiles are
        # impossible (residual stream is dense gaussian), so detect and retry.
        v = out.reshape(B, (2 * L) // 128, 128, DIM // 512, 512)
        if np.abs(v).max(axis=(2, 4)).min() > 0.0:
            return out
    return out


def kernel(**inputs) -> np.ndarray:
    return _run(_make_in_maps(inputs))


# revision 49
# speedup vs baseline: 1.2489x; 1.0578x over previous
"""AV temporal gated-conv MLP block for Trainium2 (8 NeuronCores, Bass/Tile).

Per-core strategy: pure data parallelism over the batch (B=8 -> 1 batch
element per core, both modalities on the same core since the gating couples
them). No collectives. Per core, loop over 4 l-blocks of 512 tokens:

  T: rms-norm in natural [l, d] layout -- ACT Square with accum_out for the
     square-sums (into one [128,4] stats tile per mod-block), rsqrt as a
     single BATCHED [128,4] DVE Newton chain (bit-trick seed + 3 iters; the
     chain's latency paces xT availability, so batching 4 l-tiles cuts the
     critical path 4x), per-partition scale, then DMA-XBAR transpose
     (dma_start(transpose=True)): one instruction turns xn [128(l),1024(d)]
     into the [128(d%128), 8(dc), 128(l)] xT slab -- ZERO PE involvement.
  A: in_proj as bf16 matmuls (1 cycle/row PE rate) producing [e, l] chunks;
     causal depthwise conv on the x-half as 1 tensor_scalar + 3
     scalar_tensor_tensor fused MACs on VectorE in BF16 (2x DVE rate); the
     3-col conv-history prepend/save copies run on ACT (tiny DVE ops pay
     ~+124ns/instr on real HW -- measured via microprobe); silu (ACT, bf16
     out) straight out of PSUM; cross-modal gate writes bf16 gat.
  B: out_proj bf16 matmuls against SBUF-RESIDENT out_proj weights (loaded
     once, 8MB bf16 both modalities, killing the 64MB f32 wout restream);
     residual add reads the retained natural-layout x tiles (no re-read).

The whole datapath is bf16 (tolerance 2e-2; measured rel-err 3.7e-3):
weights, x, xT, gat, y. Per-core HBM traffic drops 252MB (f32r) -> 88MB.

HW microprobes (slope-timed, this axon setup): pure PE matmul stream runs
at exactly the cost-model rate (ratio 0.992, 2.4GHz, Ldweights hidden);
DMA runs ~1.25x FASTER than model; DVE pays +124ns/instr over model; and
a PE stream with CONCURRENT DMA streaming runs 1.17x slower than model
(SBUF/fabric contention) -- which is why cutting DMA bytes and DVE
instruction count dominated the tuning. Cost model: 654us/rep marginal
(= the PE floor); HW slope-measured ~817us/exec (vs 872us baseline).
"""
import sys

if "/opt/trn_rl_repo" not in sys.path:
    sys.path.insert(0, "/opt/trn_rl_repo")

import numpy as np

DIM = 1024
INNER = 2048
L = 2048
B = 8
NCORES = 8
EPS = 1e-5
LB = 512              # l-block (tokens per block)
NB = L // LB          # 4 blocks
NXC = INNER // 128    # 16 x-half e-chunks per modality
NKD = DIM // 128      # 8 contraction chunks for in_proj
NLT = LB // 128       # 4 l-tiles per block
NN = DIM // 512       # 2 out_proj n-tiles
CONV_MODE = "dve"  # "dve" | "gps" | "dve+gps"  (gps fails in neuronxcc)

_cache = {}


def _build_nc(conv_mode=None, repeat=1):
    conv_mode = conv_mode or CONV_MODE
    from contextlib import ExitStack

    import concourse.bass as bass
    import concourse.tile as tile
    from concourse import bacc, mybir
    from concourse.masks import make_identity

    dt = mybir.dt
    f32 = dt.float32
    bf16 = dt.bfloat16
    i32 = dt.int32
    AOP = mybir.AluOpType
    AF = mybir.ActivationFunctionType

    nc = bacc.Bacc("TRN2", target_bir_lowering=False, debug=False,
                   num_devices=NCORES)

    x_dram = {
        "a": nc.dram_tensor("xa", [L, DIM], bf16, kind="ExternalInput").ap(),
        "v": nc.dram_tensor("xv", [L, DIM], bf16, kind="ExternalInput").ap(),
    }
    win_dram = {
        "a": nc.dram_tensor("wina", [2 * NXC, 128, NKD, 128], bf16,
                            kind="ExternalInput").ap(),
        "v": nc.dram_tensor("winv", [2 * NXC, 128, NKD, 128], bf16,
                            kind="ExternalInput").ap(),
    }
    wout_dram = {
        "a": nc.dram_tensor("wouta", [128, NXC * NN * 512], bf16,
                            kind="ExternalInput").ap(),
        "v": nc.dram_tensor("woutv", [128, NXC * NN * 512], bf16,
                            kind="ExternalInput").ap(),
    }
    cw_dram = {
        "a": nc.dram_tensor("cwa", [128, NXC * 4], f32, kind="ExternalInput").ap(),
        "v": nc.dram_tensor("cwv", [128, NXC * 4], f32, kind="ExternalInput").ap(),
    }
    cbc_dram = {
        "a": nc.dram_tensor("cbca", [128, NXC], f32, kind="ExternalInput").ap(),
        "v": nc.dram_tensor("cbcv", [128, NXC], f32, kind="ExternalInput").ap(),
    }
    y = nc.dram_tensor("y", [2 * L, DIM], bf16, kind="ExternalOutput").ap()

    MODS = ("a", "v")

    with tile.TileContext(nc) as tc, ExitStack() as ctx:
        sing = ctx.enter_context(tc.tile_pool(name="sing", bufs=1))
        p_xT = ctx.enter_context(tc.tile_pool(name="xT", bufs=2))
        p_gat = ctx.enter_context(tc.tile_pool(name="gat", bufs=2))
        p_xin = ctx.enter_context(tc.tile_pool(name="xin", bufs=18))
        p_xn = ctx.enter_context(tc.tile_pool(name="xn", bufs=6))
        p_stat = ctx.enter_context(tc.tile_pool(name="stat", bufs=4))
        p_win = ctx.enter_context(tc.tile_pool(name="win", bufs=6))
        p_axp = ctx.enter_context(tc.tile_pool(name="axp", bufs=4))
        p_sv = ctx.enter_context(tc.tile_pool(name="sv", bufs=3))
        p_yout = ctx.enter_context(tc.tile_pool(name="yout", bufs=4))
        p_ps = ctx.enter_context(
            tc.tile_pool(name="ps", bufs=8, space=bass.MemorySpace.PSUM))

        magic = sing.tile([128, NLT], i32, name="magic", tag="magic")
        nc.vector.memset(magic[:], 0x5F3759DF)

        cw_sb, cbc_sb, hist, wout_sb = {}, {}, {}, {}

        def setup_conv_state():
            for mod in MODS:
                cw_sb[mod] = sing.tile([128, NXC * 4], f32, name=f"cw_{mod}",
                                       tag=f"cw_{mod}")
                nc.sync.dma_start(cw_sb[mod][:], cw_dram[mod][:])
                cbc_sb[mod] = sing.tile([128, NXC], f32, name=f"cbc_{mod}",
                                        tag=f"cbc_{mod}")
                nc.sync.dma_start(cbc_sb[mod][:], cbc_dram[mod][:])
                hist[mod] = sing.tile([128, NXC * 3], bf16, name=f"hist_{mod}",
                                      tag=f"hist_{mod}")
                nc.vector.memset(hist[mod][:], 0.0)

        def load_wout():
            # split into 4 chunks per modality so no single 4MB DMA blocks a
            # later win-stream load that lands behind it in the same queue
            NSP = 4
            csz = NXC * NN * 512 // NSP
            for mod in MODS:
                wout_sb[mod] = sing.tile([128, NXC * NN * 512], bf16,
                                         name=f"wout_{mod}", tag=f"wout_{mod}")
                for s in range(NSP):
                    nc.sync.dma_start(wout_sb[mod][:, s * csz:(s + 1) * csz],
                                      wout_dram[mod][:, s * csz:(s + 1) * csz])

        def emit_T(blk, first=False, after_mod=None):
            l0 = blk * LB
            xTt, xts = {}, {}
            # schedule the load+square chain ~a half block early (priority-only:
            # the PE transposes keep their natural slot so PSUM isn't grabbed).
            # First block: strictly stagger mod-a chain > a-weights > mod-v
            # chain > v-weights so the scheduler doesn't round-robin the DMAs
            # (mod-a's chain latency gates the very first matmul).
            for mod in MODS:
                boost = 0 if first else 600
                if after_mod is not None and mod != MODS[0]:
                    after_mod(MODS[0])
                xTt[mod] = p_xT.tile([128, NKD * LB], bf16, name="xT", tag="xT")
                xts[mod] = []
                xns = []
                stats = p_stat.tile([128, NLT], f32, name="ssum", tag="ssum")
                with tc.high_priority(offset=boost):
                    for lt in range(NLT):
                        xt = p_xin.tile([128, DIM], bf16, name="xin", tag="xin")
                        nc.sync.dma_start(
                            xt[:], x_dram[mod][l0 + lt * 128: l0 + (lt + 1) * 128, :])
                        xnt = p_xn.tile([128, DIM], bf16, name="xn", tag="xn")
                        nc.scalar.activation(xnt[:], xt[:], AF.Square,
                                             accum_out=stats[:, lt:lt + 1])
                        xts[mod].append(xt)
                        xns.append(xnt)
                with tc.high_priority(offset=boost):
                    # rsqrt of all NLT square-sums in ONE batched [128, NLT]
                    # Newton chain (bit-trick seed + 3 iterations): 4x fewer
                    # serial DVE ops than per-l-tile chains -- this chain's
                    # latency paces xT availability and hence PE start
                    m = p_stat.tile([128, NLT], f32, name="mvar", tag="mvar")
                    nc.vector.tensor_scalar(m[:], stats[:], 1.0 / DIM, EPS,
                                            AOP.mult, AOP.add)
                    ts = p_stat.tile([128, NLT], f32, name="nsh", tag="nsh")
                    nc.vector.tensor_scalar(ts[:].bitcast(i32), m[:].bitcast(i32),
                                            1, None, AOP.logical_shift_right)
                    yv = p_stat.tile([128, NLT], f32, name="ny0", tag="ny0")
                    nc.vector.tensor_tensor(yv[:].bitcast(i32), magic[:],
                                            ts[:].bitcast(i32), AOP.subtract)
                    mh = p_stat.tile([128, NLT], f32, name="nmh", tag="nmh")
                    nc.vector.tensor_scalar(mh[:], m[:], -0.5, None, AOP.mult)
                    for it in range(3):
                        aa = p_stat.tile([128, NLT], f32, name="na", tag="na")
                        nc.vector.tensor_mul(aa[:], yv[:], yv[:])
                        cc = p_stat.tile([128, NLT], f32, name="ncc", tag="ncc")
                        nc.vector.tensor_tensor(cc[:], aa[:], mh[:], AOP.mult)
                        nc.vector.tensor_scalar(cc[:], cc[:], 1.0, 1.5,
                                                AOP.mult, AOP.add)
                        y2 = p_stat.tile([128, NLT], f32, name="nyi", tag="nyi")
                        nc.vector.tensor_mul(y2[:], yv[:], cc[:])
                        yv = y2
                    for lt in range(NLT):
                        nc.vector.tensor_scalar(xns[lt][:], xts[mod][lt][:],
                                                yv[:, lt:lt + 1], None, AOP.mult)
                with tc.high_priority(offset=boost):
                    for lt in range(NLT):
                        # DMA XBAR transpose: xn [128(l), 1024(d)] -> xT slab
                        # [128(d%128), 8(d//128), 128(l)] in one instruction
                        # (64 16x128 xbar tiles, ~0.9us) -- zero PE involvement
                        dst = xTt[mod].rearrange("p (dc l) -> p dc l", dc=NKD)[
                            :, :, lt * 128:(lt + 1) * 128]
                        nc.sync.dma_start(dst, xns[lt][:], transpose=True)
            if after_mod is not None:
                after_mod(MODS[1])

            return xTt, xts

        def prefetch_w(blk):
            tiles = {}
            for mod in MODS:
                for half, m in (("x", 0), ("w", NXC)):
                    wt = p_win.tile([128, DIM], bf16, name="win", tag="win")
                    nc.sync.dma_start(
                        wt[:].rearrange("p (kc m) -> p kc m", kc=NKD),
                        win_dram[mod][m])
                    tiles[(mod, half)] = wt
            return tiles

        def emit_A(blk, xTt, pre=None):
            gat = {}
            for mod in MODS:
                gat[mod] = p_gat.tile([128, NXC * LB], bf16, name="gat", tag="gat")
            for c in range(NXC):
                pp = {}
                for mod in MODS:
                    for half, m in (("x", c), ("w", NXC + c)):
                        if c == 0 and pre is not None:
                            wt = pre[(mod, half)]
                        else:
                            wt = p_win.tile([128, DIM], bf16, name="win", tag="win")
                            nc.sync.dma_start(
                                wt[:].rearrange("p (kc m) -> p kc m", kc=NKD),
                                win_dram[mod][m])
                        ps = p_ps.tile([128, LB], f32, name="ps", tag="ps")
                        for kc in range(NKD):
                            nc.tensor.matmul(
                                ps[:],
                                lhsT=wt[:, kc * 128:(kc + 1) * 128],
                                rhs=xTt[mod][:, kc * LB:(kc + 1) * LB],
                                start=(kc == 0), stop=(kc == NKD - 1))
                        pp[(mod, half)] = ps
                sv = {}
                for mod in MODS:
                    # silu straight out of PSUM, bf16 result (2x DVE gate rate)
                    s = p_sv.tile([128, LB], bf16, name="sv", tag="sv")
                    nc.scalar.activation(s[:], pp[(mod, "w")][:], AF.Silu)
                    sv[mod] = s
                for mi, (mod, other) in enumerate((("a", "v"), ("v", "a"))):
                    if conv_mode == "dve":
                        cm = "dve"
                    elif conv_mode == "gps":
                        cm = "gps"
                    elif conv_mode == "dve+gps":
                        cm = "dve" if mi == 0 else "gps"
                    else:
                        raise ValueError(conv_mode)
                    # bf16 conv: halves DVE MAC time; the tiny 3-col history
                    # prepend/save copies run on ACT, not DVE -- per-instr
                    # overhead on HW (~+124ns) makes 3-col DVE ops pure waste
                    axp = p_axp.tile([128, LB + 3], bf16, name="axp", tag="axp")
                    nc.scalar.copy(axp[:, 0:3],
                                   hist[mod][:, c * 3:(c + 1) * 3])
                    nc.scalar.copy(axp[:, 3:LB + 3], pp[(mod, "x")][:])
                    nc.scalar.copy(hist[mod][:, c * 3:(c + 1) * 3],
                                   axp[:, LB:LB + 3])
                    eng = nc.vector if cm == "dve" else nc.gpsimd
                    acc = p_sv.tile([128, LB], bf16, name="convacc", tag="convacc")
                    eng.tensor_scalar(
                        acc[:], axp[:, 0:LB],
                        cw_sb[mod][:, c * 4: c * 4 + 1],
                        cbc_sb[mod][:, c:c + 1], AOP.mult, AOP.add)
                    for t in range(1, 4):
                        acc2 = p_sv.tile([128, LB], bf16, name="convacc", tag="convacc")
                        eng.scalar_tensor_tensor(
                            acc2[:], axp[:, t:t + LB],
                            cw_sb[mod][:, c * 4 + t: c * 4 + t + 1],
                            acc[:], AOP.mult, AOP.add)
                        acc = acc2
                    nc.vector.tensor_mul(gat[mod][:, c * LB:(c + 1) * LB],
                                         acc[:], sv[other][:])

            return gat

        def emit_B(blk, gat, xts):
            l0 = blk * LB
            for mod in MODS:
                yoff = 0 if mod == "a" else L
                for n in range(NN):
                    po = [p_ps.tile([128, 512], f32, name="ps", tag="ps")
                          for _ in range(NLT)]
                    for c2 in range(NXC):
                        w = wout_sb[mod][:, (c2 * NN + n) * 512:
                                         (c2 * NN + n + 1) * 512]
                        for mt in range(NLT):
                            nc.tensor.matmul(
                                po[mt][:],
                                lhsT=gat[mod][:, c2 * LB + mt * 128:
                                              c2 * LB + (mt + 1) * 128],
                                rhs=w,
                                start=(c2 == 0), stop=(c2 == NXC - 1))
                    for mt in range(NLT):
                        yt = p_yout.tile([128, 512], bf16, name="yout", tag="yout")
                        nc.vector.tensor_add(
                            yt[:], po[mt][:],
                            xts[mod][mt][:, n * 512:(n + 1) * 512])
                        nc.sync.dma_start(
                            y[yoff + l0 + mt * 128: yoff + l0 + (mt + 1) * 128,
                              n * 512:(n + 1) * 512], yt[:])

        # Flattened (rep, blk) sequence. Per step: A(blk) is emitted, then
        # T(blk+1) and the next weight prefetch, then B(blk). The c0 weight
        # pair is prefetched before T(0) so the first matmul waits only on
        # the first xT transpose chain; conv state DMAs queue after x loads.
        nsteps = repeat * NB
        # Startup-ordered emission: mod-a's T chain first, then its c0 weight
        # tiles (so the first in_proj matmuls start as soon as xT_a lands),
        # then mod-v's chain + weights, and conv state (needed ~30us in) last.
        pre = {}

        def _prefetch_mod(mod):
            for half, m in (("x", 0), ("w", NXC)):
                wt = p_win.tile([128, DIM], bf16, name="win", tag="win")
                nc.sync.dma_start(
                    wt[:].rearrange("p (kc m) -> p kc m", kc=NKD),
                    win_dram[mod][m])
                pre[(mod, half)] = wt

        cur = emit_T(0, first=True, after_mod=_prefetch_mod)
        setup_conv_state()
        for step in range(nsteps):
            blk = step % NB
            gat = emit_A(blk, cur[0], pre)
            if step == 0:
                load_wout()
            if step + 1 < nsteps:
                pre = prefetch_w((step + 1) % NB)
                nxt = emit_T((step + 1) % NB)
            emit_B(blk, gat, cur[1])
            if step + 1 < nsteps:
                cur = nxt

    nc.finalize()
    return nc


def _get_nc(conv_mode=None, repeat=1):
    key = ("nc", conv_mode or CONV_MODE, repeat)
    if key not in _cache:
        _cache[key] = _build_nc(conv_mode, repeat)
    return _cache[key]


def _bf16(a):
    import ml_dtypes
    return np.asarray(a, np.float32).astype(ml_dtypes.bfloat16)


def _prep_weights(inputs):
    f = np.float32
    a_in = np.asarray(inputs["a_in_w"], f) * np.asarray(inputs["a_norm_w"], f)[None, :]
    v_in = np.asarray(inputs["v_in_w"], f) * np.asarray(inputs["v_norm_w"], f)[None, :]

    def pack_in(w):  # [2*INNER, DIM] -> [32, 128, 8, 128]: m-tile x [d%128, d//128, e]
        t = w.T.reshape(NKD, 128, 2 * NXC, 128)   # [kc, p, m, e]
        return _bf16(np.ascontiguousarray(t.transpose(2, 1, 0, 3)))

    def pack_out(w):  # [DIM, INNER] -> [128, NXC*NN*512]: [e%128, (e//128, d//512, d%512)]
        t = np.asarray(w, f).T.reshape(NXC, 128, NN, 512)   # [c2, p, n, d]
        return _bf16(np.ascontiguousarray(
            t.transpose(1, 0, 2, 3).reshape(128, NXC * NN * 512)))

    def pack_cw(w):  # [INNER, 1, 4] -> [128, 64]
        return np.ascontiguousarray(
            np.asarray(w, f)[:, 0, :].reshape(NXC, 128, 4)
            .transpose(1, 0, 2).reshape(128, NXC * 4))

    return {
        "wina": pack_in(a_in),
        "winv": pack_in(v_in),
        "wouta": pack_out(np.asarray(inputs["a_out_w"], f)),
        "woutv": pack_out(np.asarray(inputs["v_out_w"], f)),
        "cwa": pack_cw(inputs["a_conv_w"]),
        "cwv": pack_cw(inputs["v_conv_w"]),
        "cbca": np.ascontiguousarray(
            np.asarray(inputs["a_conv_b"], f).reshape(NXC, 128).T),
        "cbcv": np.ascontiguousarray(
            np.asarray(inputs["v_conv_b"], f).reshape(NXC, 128).T),
    }


def _make_in_maps(inputs):
    shared = _prep_weights(inputs)
    audio = _bf16(inputs["audio"])
    visual = _bf16(inputs["visual"])
    in_maps = []
    for b in range(B):
        m = dict(shared)
        m["xa"] = np.ascontiguousarray(audio[b])
        m["xv"] = np.ascontiguousarray(visual[b])
        in_maps.append(m)
    return in_maps


def _run(in_maps):
    from concourse.bass_utils import run_bass_kernel_spmd
    nc = _get_nc()
    out = None
    for _attempt in range(3):
        res = run_bass_kernel_spmd(nc, in_maps, core_ids=list(range(NCORES)))
        out = np.stack([res.results[b]["y"] for b in range(B)],
                       axis=0).astype(np.float32)
        # Rare terminal-side flake: a dropped write leaves a [128, 512] output
        # tile as the donated zero-buffer contents. Legit all-zero tiles are
        # impossible (residual stream is dense gaussian), so detect and retry.
        v = out.reshape(B, (2 * L) // 128, 128, DIM // 512, 512)
        if np.abs(v).max(axis=(2, 4)).min() > 0.0:
            return out
    return out


def kernel(**inputs) -> np.ndarray:
    return _run(_make_in_maps(inputs))
